# revision 1
# baseline (speedup 1.0000x reference)
"""GNN NodeBlock (message passing + 3-layer MLP + LayerNorm) on 8 Trainium2 cores.

Strategy (data parallel over nodes):
  - Shard 50000 nodes across 8 cores (6250 each, padded to 6272 = 49*128).
  - Host partitions edges by destination node (so segment_sum is core-local),
    groups them per 128-node tile, and lays them out in fixed-capacity slots
    (CH chunks of 128 edges per tile; CH derived from the data's max tile degree).
  - On device, per 128-node tile the segment-sum is computed as a sequence of
    CH matmuls on the TensorEngine:  aggT += ef_chunk[128e, 96].T @ onehot[128e, 128n]
    where onehot[e, n] = (dst_rel[e] == n) is built by one VectorEngine
    is_equal over a broadcast iota. Result aggT is [96 feat, nodes] "T-layout".
  - The MLP runs entirely in T-layout (features on partitions, nodes on the
    free dim) with weights stationary: h^T = W.T @ x^T, so no transposes are
    needed between layers. Node features enter pre-transposed from the host.
  - Layer 3 swaps the operands (activations stationary) to produce y in natural
    layout [128 nodes, 512 feats]; bias b3 is added with a K=1 ones-matmul.
    LayerNorm then reduces over the free dim: bn_stats/bn_aggr (VectorE) +
    sqrt (ScalarE) + reciprocal (VectorE), applied via one ScalarE activation
    with per-partition scale/bias.
  - All matmuls are bf16 inputs with fp32 PSUM accumulation (~4e-3 L2 rel err).

Everything is compiled once per (shape, CH) configuration and cached.
"""

import numpy as np
import ml_dtypes

P = 128
NODE_DIM = 512
EDGE_DIM = 96
HID = 1024
OUT = 512
N_NODES = 50000
N_EDGES = 800000
NCORES = 8
LN_EPS = 1e-5

NPC = N_NODES // NCORES          # 6250 nodes per core
T_TILES = -(-NPC // P)           # 49 node tiles per core
NPAD = T_TILES * P               # 6272
GMAX = 4                         # node tiles per super-tile (NT = 512 free dim)

BF16 = ml_dtypes.bfloat16

_CACHE: dict = {}


# ----------------------------------------------------------------------------
# Bass program
# ----------------------------------------------------------------------------

def _build_program(ch: int, apply_gamma_beta: bool):
    import concourse.bass as bass
    import concourse.bacc as bacc
    import concourse.mybir as mybir
    import concourse.tile as tile

    f32 = mybir.dt.float32
    bf16 = mybir.dt.bfloat16
    Act = mybir.ActivationFunctionType
    Alu = mybir.AluOpType

    KD = NODE_DIM // P           # 4 node-feat k-chunks
    KH = HID // P                # 8 hidden k-chunks
    MH = HID // P                # 8 hidden m-chunks
    KD1 = KD + 1                 # + 1 chunk for the 96 agg features

    nc = bacc.Bacc("TRN2", target_bir_lowering=False, debug=False)

    # inputs (per core)
    efb_d = nc.declare_dram_parameter("efb", [T_TILES, P, ch, P], bf16, isOutput=False)
    dstr_d = nc.declare_dram_parameter("dstr", [P, T_TILES, ch], bf16, isOutput=False)
    nfT_d = nc.declare_dram_parameter("nfT", [NODE_DIM, NPAD], bf16, isOutput=False)
    w1_d = nc.declare_dram_parameter("w1", [P, KD1 * MH * P], bf16, isOutput=False)
    w2_d = nc.declare_dram_parameter("w2", [P, KH * MH * P], bf16, isOutput=False)
    w3_d = nc.declare_dram_parameter("w3", [P, KH * OUT], bf16, isOutput=False)
    # cstB: b1T(MH) | b2T(MH); cstLN: gamma(OUT) | beta(OUT) | b3(OUT) | eps(1)
    cstB_d = nc.declare_dram_parameter("cstB", [P, 2 * MH], f32, isOutput=False)
    cstLN_d = nc.declare_dram_parameter("cstLN", [P, 3 * OUT + 1], f32, isOutput=False)
    cstb_d = nc.declare_dram_parameter("cstb", [P, P], bf16, isOutput=False)
    y_d = nc.declare_dram_parameter("y", [NPAD, OUT], f32, isOutput=True)

    groups = []
    t0 = 0
    while t0 < T_TILES:
        g = min(GMAX, T_TILES - t0)
        groups.append((t0, g))
        t0 += g

    with tile.TileContext(nc) as tc:
        with (
            tc.tile_pool(name="const", bufs=1) as constp,
            tc.tile_pool(name="ef", bufs=3) as efp,
            tc.tile_pool(name="oh", bufs=3) as ohp,
            tc.tile_pool(name="agg", bufs=3) as aggp,
            tc.tile_pool(name="nfx", bufs=2) as nfxp,
            tc.tile_pool(name="h1", bufs=2) as h1p,
            tc.tile_pool(name="h2", bufs=2) as h2p,
            tc.tile_pool(name="yo", bufs=3) as yop,
            tc.tile_pool(name="st", bufs=8) as stp,
            tc.tile_pool(name="psA", bufs=2, space="PSUM") as psA,
            tc.tile_pool(name="psM", bufs=4, space="PSUM") as psM,
            tc.tile_pool(name="psY", bufs=2, space="PSUM") as psY,
        ):
            # small constants first: the first group's aggregation only needs
            # these, so it starts while the (larger) weight DMAs stream in.
            # group 0's dst indices as their own tiny tile (~18KB): the first
            # onehot compare starts ~2us earlier than waiting on the full load
            g0 = groups[0][1]
            dstr0_sb = constp.tile([P, GMAX, ch], bf16)
            nc.sync.dma_start(out=dstr0_sb[:, 0:g0, :], in_=dstr_d[:, 0:g0, :])
            dstr_sb = constp.tile([P, T_TILES, ch], bf16)
            cstb_sb = constp.tile([P, P], bf16)
            nc.sync.dma_start(out=cstb_sb[:], in_=cstb_d[:, :])
            cstB_sb = constp.tile([P, 2 * MH], f32)
            nc.sync.dma_start(out=cstB_sb[:], in_=cstB_d[:, :])
            w1_sb = constp.tile([P, MH * KD1 * P], bf16)
            w2_sb = constp.tile([P, MH * KH * P], bf16)
            w3_sb = constp.tile([P, KH * OUT], bf16)
            cstLN_sb = constp.tile([P, 3 * OUT + 1], f32)

            nfT_ap = nfT_d[:, :].rearrange("(k p) n -> p k n", p=P)

            def emit_agg_subtile(t, aggT, s, split_dma=False):
                """Segment-sum of one 128-node tile into aggT[:, s*P:(s+1)*P].

                split_dma: stage the edge chunks as two independent tiles so the
                first matmuls start as soon as the first slice lands (tile-level
                dependency tracking would otherwise wait for the whole load)."""
                if split_dma and ch > 4:
                    h = 4
                    parts = [(0, h), (h, ch)]
                else:
                    parts = [(0, ch)]
                efs, ohs = [], []
                dsrc = dstr0_sb if t < groups[0][1] else dstr_sb
                for (c0, c1) in parts:
                    ef_t = efp.tile([P, c1 - c0, P], bf16, tag="ef", name="ef_t")
                    nc.sync.dma_start(out=ef_t[:], in_=efb_d[t, :, c0:c1, :])
                    oh_t = ohp.tile([P, c1 - c0, P], bf16, tag="oh", name="oh_t")
                    nc.vector.tensor_tensor(
                        out=oh_t[:],
                        in0=dsrc[:, t, c0:c1, None].to_broadcast([P, c1 - c0, P]),
                        in1=cstb_sb[:, None, 0:P].to_broadcast([P, c1 - c0, P]),
                        op=Alu.is_equal,
                    )
                    efs.append(ef_t)
                    ohs.append(oh_t)
                ps_a = psA.tile([P, P], f32, tag="psA")
                cc = 0
                for (c0, c1), ef_t, oh_t in zip(parts, efs, ohs):
                    for j in range(c1 - c0):
                        nc.tensor.matmul(
                            out=ps_a[:],
                            lhsT=ef_t[:, j, :],
                            rhs=oh_t[:, j, :],
                            start=(cc == 0),
                            stop=(cc == ch - 1),
                        )
                        cc += 1
                nc.scalar.copy(out=aggT[:, s * P:(s + 1) * P], in_=ps_a[0:EDGE_DIM, :])

            # group 0's aggregation up front (weight DMAs stream in behind it)
            agg_tiles = {}
            agg_tiles[0] = aggp.tile([EDGE_DIM, GMAX * P], bf16, tag="agg",
                                     name="aggT")
            for s in range(groups[0][1]):
                emit_agg_subtile(groups[0][0] + s, agg_tiles[0], s,
                                 split_dma=(s <= 2))

            for gi, (tstart, g) in enumerate(groups):
                nt = g * P  # free-dim width of this super-tile
                n0 = tstart * P
                aggT = agg_tiles.pop(gi)

                # ---- node features (pre-transposed on host) ----
                nfx = nfxp.tile([P, KD, GMAX * P], bf16, tag="nfx")
                nc.sync.dma_start(out=nfx[:, :, 0:nt], in_=nfT_ap[:, :, n0:n0 + nt])
                if gi == 0:
                    # per-m weight slices stream in behind group 0's agg work so
                    # layer 1/2 can begin as soon as their own slice lands
                    for m in range(MH):
                        nc.sync.dma_start(
                            out=w1_sb[:, m * KD1 * P:(m + 1) * KD1 * P],
                            in_=w1_d[:, m * KD1 * P:(m + 1) * KD1 * P])
                    # full dst-index table (needed from group 1 onward)
                    nc.sync.dma_start(out=dstr_sb[:], in_=dstr_d[:, :, :])

                # ---- layer 1: h1T[m] = relu(W1.T @ xT + b1), x = [nf; agg] ----
                h1 = h1p.tile([P, KH, GMAX * P], bf16, tag="h1")
                for m in range(MH):
                    ps = psM.tile([P, GMAX * P], f32, tag="psM")
                    for k in range(KD):
                        nc.tensor.matmul(
                            out=ps[:, 0:nt],
                            lhsT=w1_sb[:, (m * KD1 + k) * P:(m * KD1 + k + 1) * P],
                            rhs=nfx[:, k, 0:nt],
                            start=(k == 0),
                            stop=False,
                        )
                    nc.tensor.matmul(
                        out=ps[:, 0:nt],
                        lhsT=w1_sb[0:EDGE_DIM, (m * KD1 + KD) * P:(m * KD1 + KD) * P + P],
                        rhs=aggT[:, 0:nt],
                        start=False,
                        stop=True,
                    )
                    nc.scalar.activation(
                        out=h1[:, m, 0:nt], in_=ps[:, 0:nt], func=Act.Relu,
                        bias=cstB_sb[:, m:m + 1],
                    )
                    if gi == 0:
                        nc.sync.dma_start(
                            out=w2_sb[:, m * KH * P:(m + 1) * KH * P],
                            in_=w2_d[:, m * KH * P:(m + 1) * KH * P])

                # ---- layer 2 ----
                h2 = h2p.tile([P, KH, GMAX * P], bf16, tag="h2")
                for m in range(MH):
                    ps = psM.tile([P, GMAX * P], f32, tag="psM")
                    for k in range(KH):
                        nc.tensor.matmul(
                            out=ps[:, 0:nt],
                            lhsT=w2_sb[:, (m * KH + k) * P:(m * KH + k + 1) * P],
                            rhs=h1[:, k, 0:nt],
                            start=(k == 0),
                            stop=(k == KH - 1),
                        )
                    nc.scalar.activation(
                        out=h2[:, m, 0:nt], in_=ps[:, 0:nt], func=Act.Relu,
                        bias=cstB_sb[:, MH + m:MH + m + 1],
                    )
                    if gi == 0 and m < 2:
                        if m == 0:
                            nc.sync.dma_start(out=w3_sb[:], in_=w3_d[:, :])
                        else:
                            nc.sync.dma_start(out=cstLN_sb[:], in_=cstLN_d[:, :])

                # ---- layer 3 (nodes on partitions) + LayerNorm ----
                # aggregation for group gi+1 interleaves here: its matmuls keep
                # TensorE fed while each subtile's LayerNorm chain drains.
                if gi + 1 < len(groups):
                    tstart_nx, g_nx = groups[gi + 1]
                    agg_tiles[gi + 1] = aggp.tile([EDGE_DIM, GMAX * P], bf16,
                                                  tag="agg", name="aggT")
                else:
                    tstart_nx, g_nx = 0, 0
                for s in range(max(g, g_nx)):
                    if s < g_nx:
                        emit_agg_subtile(tstart_nx + s, agg_tiles[gi + 1], s)
                    if s >= g:
                        continue
                    ps_y = psY.tile([P, OUT], f32, tag="psY")
                    for k in range(KH):
                        nc.tensor.matmul(
                            out=ps_y[:],
                            lhsT=h2[:, k, s * P:(s + 1) * P],
                            rhs=w3_sb[:, k * OUT:(k + 1) * OUT],
                            start=(k == 0),
                            stop=(k == KH - 1),
                        )
                    # + b3 (broadcast rows) on VectorE, off the TensorE critical path
                    nc.vector.tensor_tensor(
                        out=ps_y[:], in0=ps_y[:],
                        in1=cstLN_sb[:, 2 * OUT:3 * OUT],
                        op=Alu.add,
                    )
                    st6 = stp.tile([P, 6], f32, tag="st6")
                    nc.vector.bn_stats(st6[:], ps_y[:])
                    mv = stp.tile([P, 2], f32, tag="mv")
                    nc.vector.bn_aggr(mv[:], st6[:])
                    std = stp.tile([P, 1], f32, tag="std")
                    nc.scalar.activation(std[:], mv[:, 1:2], Act.Sqrt,
                                         bias=cstLN_sb[:, 3 * OUT:])
                    rstd = stp.tile([P, 1], f32, tag="rstd")
                    nc.vector.reciprocal(rstd[:], std[:])
                    nmr = stp.tile([P, 1], f32, tag="nmr")
                    nc.vector.tensor_scalar(
                        out=nmr[:], in0=mv[:, 0:1], scalar1=rstd[:], scalar2=-1.0,
                        op0=Alu.mult, op1=Alu.mult,
                    )
                    yn = yop.tile([P, OUT], f32, tag="yn")
                    nc.scalar.activation(
                        out=yn[:], in_=ps_y[:], func=Act.Identity,
                        bias=nmr[:], scale=rstd[:],
                    )
                    if apply_gamma_beta:
                        nc.vector.tensor_tensor(
                            out=yn[:], in0=yn[:],
                            in1=cstLN_sb[:, 0:OUT], op=Alu.mult,
                        )
                        nc.vector.tensor_tensor(
                            out=yn[:], in0=yn[:],
                            in1=cstLN_sb[:, OUT:2 * OUT], op=Alu.add,
                        )
                    r0 = (tstart + s) * P
                    nc.sync.dma_start(out=y_d[r0:r0 + P, :], in_=yn[:])

    nc.compile()
    return nc


# ----------------------------------------------------------------------------
# Host-side sharding / layout prep
# ----------------------------------------------------------------------------

def _prep_core(c, node_feat, edge_feat, dst, ch):
    lo = c * NPC
    sel = np.flatnonzero((dst >= lo) & (dst < lo + NPC))
    d = (dst[sel] - lo).astype(np.int64)
    order = np.argsort(d, kind="stable")
    sel = sel[order]
    d = d[order]
    tile_of = d >> 7
    counts = np.bincount(tile_of, minlength=T_TILES)
    offs = np.zeros(T_TILES, np.int64)
    np.cumsum(counts[:-1], out=offs[1:])
    rank = np.arange(d.size) - offs[tile_of]
    p_slot = rank % P
    c_slot = rank // P
    assert c_slot.max(initial=0) < ch

    efb = np.zeros((T_TILES, P, ch, P), BF16)
    efb[tile_of, p_slot, c_slot, :EDGE_DIM] = edge_feat[sel].astype(BF16)
    dstr = np.full((T_TILES, P, ch), -1.0, BF16)
    dstr[tile_of, p_slot, c_slot] = (d - (tile_of << 7)).astype(BF16)
    dstr = np.ascontiguousarray(dstr.transpose(1, 0, 2))

    nfT = np.zeros((NODE_DIM, NPAD), BF16)
    nfT[:, :NPC] = node_feat[lo:lo + NPC].T.astype(BF16)
    return {"efb": efb, "dstr": dstr, "nfT": nfT}


def _prep_shared(W1, b1, W2, b2, W3, b3, gamma, beta):
    KD1 = NODE_DIM // P + 1
    MH = HID // P
    KH = HID // P

    w1p = np.zeros((KD1 * P, HID), np.float32)
    w1p[:NODE_DIM + EDGE_DIM] = W1
    # m-major: col index (m*KD1 + k)*P + j
    w1 = np.ascontiguousarray(
        w1p.reshape(KD1, P, MH, P).transpose(1, 2, 0, 3)).reshape(P, -1).astype(BF16)
    w2 = np.ascontiguousarray(
        W2.reshape(KH, P, MH, P).transpose(1, 2, 0, 3)).reshape(P, -1).astype(BF16)
    w3 = np.ascontiguousarray(
        W3.reshape(KH, P, OUT).transpose(1, 0, 2)).reshape(P, -1).astype(BF16)

    cstB = np.ascontiguousarray(np.concatenate(
        [b1.reshape(MH, P).T, b2.reshape(MH, P).T], axis=1).astype(np.float32))
    cstLN = np.ascontiguousarray(np.concatenate([
        np.tile(gamma.reshape(1, OUT), (P, 1)),
        np.tile(beta.reshape(1, OUT), (P, 1)),
        np.tile(b3.reshape(1, OUT), (P, 1)),
        np.full((P, 1), LN_EPS, np.float32),
    ], axis=1).astype(np.float32))

    cstb = np.tile(np.arange(P, dtype=np.float32)[None, :], (P, 1)).astype(BF16)
    return {"w1": w1, "w2": w2, "w3": w3, "cstB": cstB, "cstLN": cstLN, "cstb": cstb}


# ----------------------------------------------------------------------------
# Entry point
# ----------------------------------------------------------------------------

def _ensure_axon_hooks_importable():
    """bass_utils imports antenv.axon_hooks when tracing is requested (even via
    the BASS_TRACE env var); provide a no-op stub if the module is absent so
    that path degrades to trace-skipped instead of crashing."""
    try:
        import antenv.axon_hooks  # noqa: F401
    except Exception:
        import sys
        import types
        try:
            import antenv
        except Exception:
            return
        mod = types.ModuleType('antenv.axon_hooks')
        mod._hook = None
        mod.set_axon_ntff_profile_hook = lambda h: setattr(mod, '_hook', h)
        mod.get_axon_ntff_profile_hook = lambda: mod._hook
        sys.modules['antenv.axon_hooks'] = mod
        antenv.axon_hooks = mod


def kernel(node_feat, edge_feat, edge_index, n_nodes, W1, b1, W2, b2, W3, b3,
           gamma, beta, _want_trace=False):
    from concourse.bass_utils import run_bass_kernel_spmd
    _ensure_axon_hooks_importable()

    node_feat = np.asarray(node_feat, dtype=np.float32)
    edge_feat = np.asarray(edge_feat, dtype=np.float32)
    edge_index = np.asarray(edge_index)
    assert int(n_nodes) == N_NODES
    assert node_feat.shape == (N_NODES, NODE_DIM)
    assert edge_feat.shape == (N_EDGES, EDGE_DIM)

    dst = edge_index[1].astype(np.int64)

    # fixed per-tile edge capacity (multiple of 128), global across cores
    counts = np.bincount(dst, minlength=N_NODES)
    padded = np.zeros((NCORES, NPAD), np.int64)
    padded[:, :NPC] = counts.reshape(NCORES, NPC)
    max_tile = int(padded.reshape(NCORES, T_TILES, P).sum(axis=2).max())
    ch = max(1, -(-max_tile // P))

    gamma = np.asarray(gamma, dtype=np.float32)
    beta = np.asarray(beta, dtype=np.float32)
    apply_gb = not (np.all(gamma == 1.0) and np.all(beta == 0.0))

    key = (ch, apply_gb)
    if key not in _CACHE:
        _CACHE[key] = _build_program(ch, apply_gb)
    nc = _CACHE[key]

    shared = _prep_shared(
        np.asarray(W1, np.float32), np.asarray(b1, np.float32),
        np.asarray(W2, np.float32), np.asarray(b2, np.float32),
        np.asarray(W3, np.float32), np.asarray(b3, np.float32),
        gamma, beta)

    in_maps = []
    for c in range(NCORES):
        m = _prep_core(c, node_feat, edge_feat, dst, ch)
        m.update(shared)
        in_maps.append(m)

    res = run_bass_kernel_spmd(nc, in_maps, list(range(NCORES)), trace=_want_trace)

    y = np.concatenate([res.results[c]["y"][:NPC] for c in range(NCORES)], axis=0)
    out = np.ascontiguousarray(y, dtype=np.float32)
    if _want_trace:
        kernel.last_results = res
    return out


kernel.last_results = None



# revision 3
# speedup vs baseline: 1.1840x; 1.1840x over previous
"""GNN NodeBlock (message passing + 3-layer MLP + LayerNorm) on 8 Trainium2 cores.

Strategy (data parallel over nodes):
  - Shard 50000 nodes across 8 cores (6250 each, padded to 6272 = 49*128).
  - Host partitions edges by destination node (so segment_sum is core-local),
    groups them per 128-node tile, and lays them out in fixed-capacity slots
    (CH chunks of 128 edges per tile; CH derived from the data's max tile degree).
  - On device, per 128-node tile the segment-sum is computed as a sequence of
    CH matmuls on the TensorEngine:  aggT += ef_chunk[128e, 96].T @ onehot[128e, 128n]
    where onehot[e, n] = (dst_rel[e] == n) is built by one VectorEngine
    is_equal over a broadcast iota. Result aggT is [96 feat, nodes] "T-layout".
  - The MLP runs entirely in T-layout (features on partitions, nodes on the
    free dim) with weights stationary: h^T = W.T @ x^T, so no transposes are
    needed between layers. Node features enter pre-transposed from the host.
  - Layer 3 swaps the operands (activations stationary) to produce y in natural
    layout [128 nodes, 512 feats]; bias b3 is added with a K=1 ones-matmul
    (last tile) or a VectorE add (other tiles, off the TensorE critical path).
    LayerNorm then reduces over the free dim: bn_stats/bn_aggr (VectorE) +
    fused rsqrt (ScalarE), applied via one ScalarE activation with
    per-partition scale/bias.
  - All matmuls are bf16 inputs with fp32 PSUM accumulation (~4e-3 L2 rel err).

Schedule details (from perfetto trace analysis):
  - ~32 zero matmuls are issued during the otherwise-dead framework preamble so
    the PE HAM clock-gate reaches 8/8 (2.4 GHz) before the real stream starts
    (otherwise the first ~12us of matmuls run at 1.2 GHz).
  - w1 slice DMAs issue on the ScalarE queue (a second HWDGE) in parallel with
    the Sync queue's agg-critical DMAs; all remaining weight DMAs are hoisted
    ahead of the group loop so no matmul ever waits on a weight.

Everything is compiled once per (shape, CH) configuration and cached.
"""

import numpy as np
import ml_dtypes

P = 128
NODE_DIM = 512
EDGE_DIM = 96
HID = 1024
OUT = 512
N_NODES = 50000
N_EDGES = 800000
NCORES = 8
LN_EPS = 1e-5

NPC = N_NODES // NCORES          # 6250 nodes per core
T_TILES = -(-NPC // P)           # 49 node tiles per core
NPAD = T_TILES * P               # 6272
GMAX = 4                         # node tiles per super-tile (NT = 512 free dim)
NWARM = 32                       # HAM warm-up matmuls during preamble

BF16 = ml_dtypes.bfloat16

_CACHE: dict = {}


# ----------------------------------------------------------------------------
# Bass program
# ----------------------------------------------------------------------------

def _build_program(ch: int, apply_gamma_beta: bool):
    import concourse.bass as bass
    import concourse.bacc as bacc
    import concourse.mybir as mybir
    import concourse.tile as tile

    f32 = mybir.dt.float32
    bf16 = mybir.dt.bfloat16
    Act = mybir.ActivationFunctionType
    Alu = mybir.AluOpType

    KD = NODE_DIM // P           # 4 node-feat k-chunks
    KH = HID // P                # 8 hidden k-chunks
    MH = HID // P                # 8 hidden m-chunks
    KD1 = KD + 1                 # + 1 chunk for the 96 agg features

    nc = bacc.Bacc("TRN2", target_bir_lowering=False, debug=False)

    # inputs (per core)
    efb_d = nc.declare_dram_parameter("efb", [T_TILES, P, ch, P], bf16, isOutput=False)
    dstr_d = nc.declare_dram_parameter("dstr", [P, T_TILES, ch], bf16, isOutput=False)
    nfT_d = nc.declare_dram_parameter("nfT", [NODE_DIM, NPAD], bf16, isOutput=False)
    w1_d = nc.declare_dram_parameter("w1", [P, KD1 * MH * P], bf16, isOutput=False)
    w2_d = nc.declare_dram_parameter("w2", [P, KH * MH * P], bf16, isOutput=False)
    w3_d = nc.declare_dram_parameter("w3", [P, KH * OUT], bf16, isOutput=False)
    # cstB: b1T(MH) | b2T(MH); cstLN: gamma(OUT) | beta(OUT) | b3(OUT) | eps(1)
    cstB_d = nc.declare_dram_parameter("cstB", [P, 2 * MH], f32, isOutput=False)
    cstLN_d = nc.declare_dram_parameter("cstLN", [P, 3 * OUT + 1], f32, isOutput=False)
    cstb_d = nc.declare_dram_parameter("cstb", [P, P], bf16, isOutput=False)
    cstb3_d = nc.declare_dram_parameter("cstb3", [1, OUT], bf16, isOutput=False)
    y_d = nc.declare_dram_parameter("y", [NPAD, OUT], f32, isOutput=True)

    groups = []
    t0 = 0
    while t0 < T_TILES:
        g = min(GMAX, T_TILES - t0)
        groups.append((t0, g))
        t0 += g

    with tile.TileContext(nc) as tc:
        with (
            tc.tile_pool(name="const", bufs=1) as constp,
            tc.tile_pool(name="warm", bufs=1) as warmp,
            tc.tile_pool(name="ef", bufs=3) as efp,
            tc.tile_pool(name="oh", bufs=3) as ohp,
            tc.tile_pool(name="agg", bufs=3) as aggp,
            tc.tile_pool(name="nfx", bufs=2) as nfxp,
            tc.tile_pool(name="h1", bufs=2) as h1p,
            tc.tile_pool(name="h2", bufs=2) as h2p,
            tc.tile_pool(name="yo", bufs=3) as yop,
            tc.tile_pool(name="st", bufs=8) as stp,
            tc.tile_pool(name="psA", bufs=3, space="PSUM") as psA,
            tc.tile_pool(name="psM", bufs=3, space="PSUM") as psM,
            tc.tile_pool(name="psY", bufs=2, space="PSUM") as psY,
        ):
            # ---- PE warm-up: zero matmuls with no DMA dependencies, issued
            # while the framework preamble / first DMAs run. Keeps the HAM
            # activity window busy so the 2.4 GHz clock gate opens before the
            # real matmul stream arrives.
            wt_a = warmp.tile([P, P], bf16)
            nc.gpsimd.memset(wt_a[:], 0.0)
            wt_b = warmp.tile([P, P], bf16)
            nc.gpsimd.memset(wt_b[:], 0.0)
            ones1 = warmp.tile([1, P], bf16)
            nc.gpsimd.memset(ones1[:], 1.0)
            ps_w = psA.tile([P, P], f32, tag="psA")
            for i in range(NWARM):
                nc.tensor.matmul(
                    out=ps_w[:], lhsT=wt_a[:], rhs=wt_b[:],
                    start=(i == 0), stop=(i == NWARM - 1),
                )

            # small constants first on the Sync queue: the first group's
            # aggregation only needs these.
            g0 = groups[0][1]
            dstr0_sb = constp.tile([P, GMAX, ch], bf16)
            nc.sync.dma_start(out=dstr0_sb[:, 0:g0, :], in_=dstr_d[:, 0:g0, :])
            cstb_sb = constp.tile([P, P], bf16)
            nc.sync.dma_start(out=cstb_sb[:], in_=cstb_d[:, :])
            b3r_sb = constp.tile([1, OUT], bf16)
            nc.sync.dma_start(out=b3r_sb[:], in_=cstb3_d[:, :])
            cstB_sb = constp.tile([P, 2 * MH], f32)
            nc.sync.dma_start(out=cstB_sb[:], in_=cstB_d[:, :])

            dstr_sb = constp.tile([P, T_TILES, ch], bf16)
            w1_sb = constp.tile([P, MH * KD1 * P], bf16)
            w2_sb = constp.tile([P, MH * KH * P], bf16)
            w3_sb = constp.tile([P, KH * OUT], bf16)
            cstLN_sb = constp.tile([P, 3 * OUT + 1], f32)

            # w1 slices stream on the ScalarE DMA queue, concurrent with the
            # Sync queue's edge-chunk DMAs (each descriptor costs ~650ns of
            # issue time on its queue; two queues halve the serialization).
            for m in range(MH):
                nc.scalar.dma_start(
                    out=w1_sb[:, m * KD1 * P:(m + 1) * KD1 * P],
                    in_=w1_d[:, m * KD1 * P:(m + 1) * KD1 * P])

            nfT_ap = nfT_d[:, :].rearrange("(k p) n -> p k n", p=P)

            def emit_agg_subtile(t, aggT, s, split_dma=False):
                """Segment-sum of one 128-node tile into aggT[:, s*P:(s+1)*P].

                split_dma: stage the edge chunks as two independent tiles so the
                first matmuls start as soon as the first slice lands (tile-level
                dependency tracking would otherwise wait for the whole load)."""
                if split_dma and ch > 4:
                    h = 4
                    parts = [(0, h), (h, ch)]
                else:
                    parts = [(0, ch)]
                efs, ohs = [], []
                dsrc = dstr0_sb if t < groups[0][1] else dstr_sb
                for (c0, c1) in parts:
                    ef_t = efp.tile([P, c1 - c0, P], bf16, tag="ef", name="ef_t")
                    nc.sync.dma_start(out=ef_t[:], in_=efb_d[t, :, c0:c1, :])
                    oh_t = ohp.tile([P, c1 - c0, P], bf16, tag="oh", name="oh_t")
                    nc.vector.tensor_tensor(
                        out=oh_t[:],
                        in0=dsrc[:, t, c0:c1, None].to_broadcast([P, c1 - c0, P]),
                        in1=cstb_sb[:, None, 0:P].to_broadcast([P, c1 - c0, P]),
                        op=Alu.is_equal,
                    )
                    efs.append(ef_t)
                    ohs.append(oh_t)
                ps_a = psA.tile([P, P], f32, tag="psA")
                cc = 0
                for (c0, c1), ef_t, oh_t in zip(parts, efs, ohs):
                    for j in range(c1 - c0):
                        nc.tensor.matmul(
                            out=ps_a[:],
                            lhsT=ef_t[:, j, :],
                            rhs=oh_t[:, j, :],
                            start=(cc == 0),
                            stop=(cc == ch - 1),
                        )
                        cc += 1
                nc.scalar.copy(out=aggT[:, s * P:(s + 1) * P], in_=ps_a[0:EDGE_DIM, :])

            # group 0's aggregation up front (weight DMAs stream in behind it)
            agg_tiles = {}
            agg_tiles[0] = aggp.tile([EDGE_DIM, GMAX * P], bf16, tag="agg",
                                     name="aggT")
            for s in range(g0):
                emit_agg_subtile(groups[0][0] + s, agg_tiles[0], s,
                                 split_dma=(s == 0))

            # prefetch everything else on the Sync queue, in need-order:
            # nfx (L1 g0) -> full dst table (g1 agg) -> w2 (L2 g0) -> w3 /
            # LN consts (L3 g0). All land well before their first consumer.
            nfx0 = nfxp.tile([P, KD, GMAX * P], bf16, tag="nfx")
            nc.sync.dma_start(out=nfx0[:, :, 0:g0 * P], in_=nfT_ap[:, :, 0:g0 * P])
            nc.sync.dma_start(out=dstr_sb[:], in_=dstr_d[:, :, :])
            for m in range(MH):
                nc.sync.dma_start(
                    out=w2_sb[:, m * KH * P:(m + 1) * KH * P],
                    in_=w2_d[:, m * KH * P:(m + 1) * KH * P])
            nc.sync.dma_start(out=w3_sb[:], in_=w3_d[:, :])
            nc.sync.dma_start(out=cstLN_sb[:], in_=cstLN_d[:, :])

            for gi, (tstart, g) in enumerate(groups):
                nt = g * P  # free-dim width of this super-tile
                n0 = tstart * P
                aggT = agg_tiles.pop(gi)
                last_group = gi == len(groups) - 1

                # ---- node features (pre-transposed on host) ----
                if gi == 0:
                    nfx = nfx0
                else:
                    nfx = nfxp.tile([P, KD, GMAX * P], bf16, tag="nfx")
                    nc.sync.dma_start(out=nfx[:, :, 0:nt],
                                      in_=nfT_ap[:, :, n0:n0 + nt])

                # ---- layer 1: h1T[m] = relu(W1.T @ xT + b1), x = [nf; agg] ----
                h1 = h1p.tile([P, KH, GMAX * P], bf16, tag="h1")
                for m in range(MH):
                    ps = psM.tile([P, GMAX * P], f32, tag="psM")
                    for k in range(KD):
                        nc.tensor.matmul(
                            out=ps[:, 0:nt],
                            lhsT=w1_sb[:, (m * KD1 + k) * P:(m * KD1 + k + 1) * P],
                            rhs=nfx[:, k, 0:nt],
                            start=(k == 0),
                            stop=False,
                        )
                    nc.tensor.matmul(
                        out=ps[:, 0:nt],
                        lhsT=w1_sb[0:EDGE_DIM, (m * KD1 + KD) * P:(m * KD1 + KD) * P + P],
                        rhs=aggT[:, 0:nt],
                        start=False,
                        stop=True,
                    )
                    nc.scalar.activation(
                        out=h1[:, m, 0:nt], in_=ps[:, 0:nt], func=Act.Relu,
                        bias=cstB_sb[:, m:m + 1],
                    )

                # ---- layer 2 ----
                h2 = h2p.tile([P, KH, GMAX * P], bf16, tag="h2")
                for m in range(MH):
                    ps = psM.tile([P, GMAX * P], f32, tag="psM")
                    for k in range(KH):
                        nc.tensor.matmul(
                            out=ps[:, 0:nt],
                            lhsT=w2_sb[:, (m * KH + k) * P:(m * KH + k + 1) * P],
                            rhs=h1[:, k, 0:nt],
                            start=(k == 0),
                            stop=(k == KH - 1),
                        )
                    nc.scalar.activation(
                        out=h2[:, m, 0:nt], in_=ps[:, 0:nt], func=Act.Relu,
                        bias=cstB_sb[:, MH + m:MH + m + 1],
                    )

                # ---- layer 3 (nodes on partitions) + LayerNorm ----
                # aggregation for group gi+1 interleaves here: its matmuls keep
                # TensorE fed while each subtile's LayerNorm chain drains.
                if gi + 1 < len(groups):
                    tstart_nx, g_nx = groups[gi + 1]
                    agg_tiles[gi + 1] = aggp.tile([EDGE_DIM, GMAX * P], bf16,
                                                  tag="agg", name="aggT")
                else:
                    tstart_nx, g_nx = 0, 0
                for s in range(max(g, g_nx)):
                    if s < g_nx:
                        emit_agg_subtile(tstart_nx + s, agg_tiles[gi + 1], s)
                    if s >= g:
                        continue
                    ps_y = psY.tile([P, OUT], f32, tag="psY")
                    if last_group:
                        # + b3 via a K=1 ones-matmul: keeps the bias add off
                        # the serial post-matmul tail of the final tile.
                        nc.tensor.matmul(
                            out=ps_y[:], lhsT=ones1[0:1, :], rhs=b3r_sb[0:1, :],
                            start=True, stop=False,
                        )
                    for k in range(KH):
                        nc.tensor.matmul(
                            out=ps_y[:],
                            lhsT=h2[:, k, s * P:(s + 1) * P],
                            rhs=w3_sb[:, k * OUT:(k + 1) * OUT],
                            start=(k == 0 and not last_group),
                            stop=(k == KH - 1),
                        )
                    if not last_group:
                        # + b3 (broadcast rows) on VectorE, off the TensorE
                        # critical path
                        nc.vector.tensor_tensor(
                            out=ps_y[:], in0=ps_y[:],
                            in1=cstLN_sb[:, 2 * OUT:3 * OUT],
                            op=Alu.add,
                        )
                    st6 = stp.tile([P, 6], f32, tag="st6")
                    nc.vector.bn_stats(st6[:], ps_y[:])
                    mv = stp.tile([P, 2], f32, tag="mv")
                    nc.vector.bn_aggr(mv[:], st6[:])
                    std = stp.tile([P, 1], f32, tag="std")
                    nc.scalar.activation(std[:], mv[:, 1:2], Act.Sqrt,
                                         bias=cstLN_sb[:, 3 * OUT:])
                    rstd = stp.tile([P, 1], f32, tag="rstd")
                    nc.vector.reciprocal(rstd[:], std[:])
                    nmr = stp.tile([P, 1], f32, tag="nmr")
                    nc.vector.tensor_scalar(
                        out=nmr[:], in0=mv[:, 0:1], scalar1=rstd[:], scalar2=-1.0,
                        op0=Alu.mult, op1=Alu.mult,
                    )
                    yn = yop.tile([P, OUT], f32, tag="yn")
                    nc.scalar.activation(
                        out=yn[:], in_=ps_y[:], func=Act.Identity,
                        bias=nmr[:], scale=rstd[:],
                    )
                    if apply_gamma_beta:
                        nc.vector.tensor_tensor(
                            out=yn[:], in0=yn[:],
                            in1=cstLN_sb[:, 0:OUT], op=Alu.mult,
                        )
                        nc.vector.tensor_tensor(
                            out=yn[:], in0=yn[:],
                            in1=cstLN_sb[:, OUT:2 * OUT], op=Alu.add,
                        )
                    r0 = (tstart + s) * P
                    nc.sync.dma_start(out=y_d[r0:r0 + P, :], in_=yn[:])

    nc.compile()
    return nc


# ----------------------------------------------------------------------------
# Host-side sharding / layout prep
# ----------------------------------------------------------------------------

def _prep_core(c, node_feat, edge_feat, dst, ch):
    lo = c * NPC
    sel = np.flatnonzero((dst >= lo) & (dst < lo + NPC))
    d = (dst[sel] - lo).astype(np.int64)
    order = np.argsort(d, kind="stable")
    sel = sel[order]
    d = d[order]
    tile_of = d >> 7
    counts = np.bincount(tile_of, minlength=T_TILES)
    offs = np.zeros(T_TILES, np.int64)
    np.cumsum(counts[:-1], out=offs[1:])
    rank = np.arange(d.size) - offs[tile_of]
    p_slot = rank % P
    c_slot = rank // P
    assert c_slot.max(initial=0) < ch

    efb = np.zeros((T_TILES, P, ch, P), BF16)
    efb[tile_of, p_slot, c_slot, :EDGE_DIM] = edge_feat[sel].astype(BF16)
    dstr = np.full((T_TILES, P, ch), -1.0, BF16)
    dstr[tile_of, p_slot, c_slot] = (d - (tile_of << 7)).astype(BF16)
    dstr = np.ascontiguousarray(dstr.transpose(1, 0, 2))

    nfT = np.zeros((NODE_DIM, NPAD), BF16)
    nfT[:, :NPC] = node_feat[lo:lo + NPC].T.astype(BF16)
    return {"efb": efb, "dstr": dstr, "nfT": nfT}


def _prep_shared(W1, b1, W2, b2, W3, b3, gamma, beta):
    KD1 = NODE_DIM // P + 1
    MH = HID // P
    KH = HID // P

    w1p = np.zeros((KD1 * P, HID), np.float32)
    w1p[:NODE_DIM + EDGE_DIM] = W1
    # m-major: col index (m*KD1 + k)*P + j
    w1 = np.ascontiguousarray(
        w1p.reshape(KD1, P, MH, P).transpose(1, 2, 0, 3)).reshape(P, -1).astype(BF16)
    w2 = np.ascontiguousarray(
        W2.reshape(KH, P, MH, P).transpose(1, 2, 0, 3)).reshape(P, -1).astype(BF16)
    w3 = np.ascontiguousarray(
        W3.reshape(KH, P, OUT).transpose(1, 0, 2)).reshape(P, -1).astype(BF16)

    cstB = np.ascontiguousarray(np.concatenate(
        [b1.reshape(MH, P).T, b2.reshape(MH, P).T], axis=1).astype(np.float32))
    cstLN = np.ascontiguousarray(np.concatenate([
        np.tile(gamma.reshape(1, OUT), (P, 1)),
        np.tile(beta.reshape(1, OUT), (P, 1)),
        np.tile(b3.reshape(1, OUT), (P, 1)),
        np.full((P, 1), LN_EPS, np.float32),
    ], axis=1).astype(np.float32))

    cstb = np.tile(np.arange(P, dtype=np.float32)[None, :], (P, 1)).astype(BF16)
    cstb3 = np.ascontiguousarray(b3.reshape(1, OUT)).astype(BF16)
    return {"w1": w1, "w2": w2, "w3": w3, "cstB": cstB, "cstLN": cstLN,
            "cstb": cstb, "cstb3": cstb3}


# ----------------------------------------------------------------------------
# Entry point
# ----------------------------------------------------------------------------

def _ensure_axon_hooks_importable():
    """bass_utils imports antenv.axon_hooks when tracing is requested (even via
    the BASS_TRACE env var); provide a no-op stub if the module is absent so
    that path degrades to trace-skipped instead of crashing."""
    try:
        import antenv.axon_hooks  # noqa: F401
    except Exception:
        import sys
        import types
        try:
            import antenv
        except Exception:
            return
        mod = types.ModuleType('antenv.axon_hooks')
        mod._hook = None
        mod.set_axon_ntff_profile_hook = lambda h: setattr(mod, '_hook', h)
        mod.get_axon_ntff_profile_hook = lambda: mod._hook
        sys.modules['antenv.axon_hooks'] = mod
        antenv.axon_hooks = mod


def kernel(node_feat, edge_feat, edge_index, n_nodes, W1, b1, W2, b2, W3, b3,
           gamma, beta, _want_trace=False):
    from concourse.bass_utils import run_bass_kernel_spmd
    _ensure_axon_hooks_importable()

    node_feat = np.asarray(node_feat, dtype=np.float32)
    edge_feat = np.asarray(edge_feat, dtype=np.float32)
    edge_index = np.asarray(edge_index)
    assert int(n_nodes) == N_NODES
    assert node_feat.shape == (N_NODES, NODE_DIM)
    assert edge_feat.shape == (N_EDGES, EDGE_DIM)

    dst = edge_index[1].astype(np.int64)

    # fixed per-tile edge capacity (multiple of 128), global across cores
    counts = np.bincount(dst, minlength=N_NODES)
    padded = np.zeros((NCORES, NPAD), np.int64)
    padded[:, :NPC] = counts.reshape(NCORES, NPC)
    max_tile = int(padded.reshape(NCORES, T_TILES, P).sum(axis=2).max())
    ch = max(1, -(-max_tile // P))

    gamma = np.asarray(gamma, dtype=np.float32)
    beta = np.asarray(beta, dtype=np.float32)
    apply_gb = not (np.all(gamma == 1.0) and np.all(beta == 0.0))

    key = (ch, apply_gb)
    if key not in _CACHE:
        _CACHE[key] = _build_program(ch, apply_gb)
    nc = _CACHE[key]

    shared = _prep_shared(
        np.asarray(W1, np.float32), np.asarray(b1, np.float32),
        np.asarray(W2, np.float32), np.asarray(b2, np.float32),
        np.asarray(W3, np.float32), np.asarray(b3, np.float32),
        gamma, beta)

    in_maps = []
    for c in range(NCORES):
        m = _prep_core(c, node_feat, edge_feat, dst, ch)
        m.update(shared)
        in_maps.append(m)

    res = run_bass_kernel_spmd(nc, in_maps, list(range(NCORES)), trace=_want_trace)

    y = np.concatenate([res.results[c]["y"][:NPC] for c in range(NCORES)], axis=0)
    out = np.ascontiguousarray(y, dtype=np.float32)
    if _want_trace:
        kernel.last_results = res
    return out


kernel.last_results = None


# revision 7
# speedup vs baseline: 1.1914x; 1.0063x over previous
"""GNN NodeBlock (message passing + 3-layer MLP + LayerNorm) on 8 Trainium2 cores.

Strategy (data parallel over nodes):
  - Shard 50000 nodes across 8 cores (6250 each, padded to 6272 = 49*128).
  - Host partitions edges by destination node (so segment_sum is core-local),
    groups them per 128-node tile, and lays them out in fixed-capacity slots
    (CH chunks of 128 edges per tile; CH derived from the data's max tile degree).
  - On device, per 128-node tile the segment-sum is computed as a sequence of
    CH matmuls on the TensorEngine:  aggT += ef_chunk[128e, 96].T @ onehot[128e, 128n]
    where onehot[e, n] = (dst_rel[e] == n) is built by one VectorEngine
    is_equal over a broadcast iota. Result aggT is [96 feat, nodes] "T-layout".
  - The MLP runs entirely in T-layout (features on partitions, nodes on the
    free dim) with weights stationary: h^T = W.T @ x^T, so no transposes are
    needed between layers. Node features enter pre-transposed from the host.
  - Layer 3 swaps the operands (activations stationary) to produce y in natural
    layout [128 nodes, 512 feats]; bias b3 is added with a K=1 ones-matmul
    (last tile) or a VectorE add (other tiles, off the TensorE critical path).
    LayerNorm then reduces over the free dim: bn_stats/bn_aggr (VectorE) +
    fused rsqrt (ScalarE), applied via one ScalarE activation with
    per-partition scale/bias.
  - All matmuls are bf16 inputs with fp32 PSUM accumulation (~4e-3 L2 rel err).

Schedule details (from perfetto trace analysis):
  - ~32 zero matmuls are issued during the otherwise-dead framework preamble so
    the PE HAM clock-gate reaches 8/8 (2.4 GHz) before the real stream starts
    (otherwise the first ~12us of matmuls run at 1.2 GHz).
  - w1 slice DMAs issue on the ScalarE queue (a second HWDGE) in parallel with
    the Sync queue's agg-critical DMAs; all remaining weight DMAs are hoisted
    ahead of the group loop so no matmul ever waits on a weight.

Everything is compiled once per (shape, CH) configuration and cached.
"""

import numpy as np
import ml_dtypes

P = 128
NODE_DIM = 512
EDGE_DIM = 96
HID = 1024
OUT = 512
N_NODES = 50000
N_EDGES = 800000
NCORES = 8
LN_EPS = 1e-5

NPC = N_NODES // NCORES          # 6250 nodes per core
T_TILES = -(-NPC // P)           # 49 node tiles per core
NPAD = T_TILES * P               # 6272
GMAX = 4                         # node tiles per super-tile (NT = 512 free dim)
NWARM = 48                       # HAM warm-up matmuls during preamble

BF16 = ml_dtypes.bfloat16

_CACHE: dict = {}


# ----------------------------------------------------------------------------
# Bass program
# ----------------------------------------------------------------------------

def _build_program(ch: int, apply_gamma_beta: bool):
    import concourse.bass as bass
    import concourse.bacc as bacc
    import concourse.mybir as mybir
    import concourse.tile as tile

    f32 = mybir.dt.float32
    bf16 = mybir.dt.bfloat16
    Act = mybir.ActivationFunctionType
    Alu = mybir.AluOpType

    KD = NODE_DIM // P           # 4 node-feat k-chunks
    KH = HID // P                # 8 hidden k-chunks
    MH = HID // P                # 8 hidden m-chunks
    KD1 = KD + 1                 # + 1 chunk for the 96 agg features

    nc = bacc.Bacc("TRN2", target_bir_lowering=False, debug=False)

    # inputs (per core)
    efb_d = nc.declare_dram_parameter("efb", [T_TILES, P, ch, P], bf16, isOutput=False)
    dstr_d = nc.declare_dram_parameter("dstr", [P, T_TILES, ch], bf16, isOutput=False)
    nfT_d = nc.declare_dram_parameter("nfT", [NODE_DIM, NPAD], bf16, isOutput=False)
    w1_d = nc.declare_dram_parameter("w1", [P, KD1 * MH * P], bf16, isOutput=False)
    w2_d = nc.declare_dram_parameter("w2", [P, KH * MH * P], bf16, isOutput=False)
    w3_d = nc.declare_dram_parameter("w3", [P, KH * OUT], bf16, isOutput=False)
    # cstB: b1T(MH) | b2T(MH); cstLN: gamma(OUT) | beta(OUT) | b3(OUT) | eps(1)
    cstB_d = nc.declare_dram_parameter("cstB", [P, 2 * MH], f32, isOutput=False)
    cstLN_d = nc.declare_dram_parameter("cstLN", [P, 3 * OUT + 1], f32, isOutput=False)
    cstb_d = nc.declare_dram_parameter("cstb", [P, P], bf16, isOutput=False)
    cstb3_d = nc.declare_dram_parameter("cstb3", [1, OUT], bf16, isOutput=False)
    y_d = nc.declare_dram_parameter("y", [NPAD, OUT], f32, isOutput=True)

    groups = []
    t0 = 0
    while t0 < T_TILES:
        g = min(GMAX, T_TILES - t0)
        groups.append((t0, g))
        t0 += g

    with tile.TileContext(nc) as tc:
        with (
            tc.tile_pool(name="const", bufs=1) as constp,
            tc.tile_pool(name="warm", bufs=1) as warmp,
            tc.tile_pool(name="ef", bufs=6) as efp,
            tc.tile_pool(name="oh", bufs=6) as ohp,
            tc.tile_pool(name="agg", bufs=3) as aggp,
            tc.tile_pool(name="nfx", bufs=2) as nfxp,
            tc.tile_pool(name="h1", bufs=2) as h1p,
            tc.tile_pool(name="h2", bufs=2) as h2p,
            tc.tile_pool(name="yo", bufs=3) as yop,
            tc.tile_pool(name="st", bufs=8) as stp,
            tc.tile_pool(name="psA", bufs=3, space="PSUM") as psA,
            tc.tile_pool(name="psM", bufs=3, space="PSUM") as psM,
            tc.tile_pool(name="psY", bufs=2, space="PSUM") as psY,
        ):
            # ---- PE warm-up: zero matmuls with no DMA dependencies, issued
            # while the framework preamble / first DMAs run. Keeps the HAM
            # activity window busy so the 2.4 GHz clock gate opens before the
            # real matmul stream arrives.
            wt_a = warmp.tile([P, P], bf16)
            nc.gpsimd.memset(wt_a[:], 0.0)
            wt_b = warmp.tile([P, P], bf16)
            nc.gpsimd.memset(wt_b[:], 0.0)
            ones1 = warmp.tile([1, P], bf16)
            nc.gpsimd.memset(ones1[:], 1.0)
            ps_w = psA.tile([P, P], f32, tag="psA")
            for i in range(NWARM):
                nc.tensor.matmul(
                    out=ps_w[:], lhsT=wt_a[:], rhs=wt_b[:],
                    start=(i == 0), stop=(i == NWARM - 1),
                )

            # small agg-control constants first on the Sync queue: the first
            # group's aggregation only needs these (+ its edge chunks).
            g0 = groups[0][1]
            dstr0_sb = constp.tile([P, GMAX, ch], bf16)
            nc.sync.dma_start(out=dstr0_sb[:, 0:g0, :], in_=dstr_d[:, 0:g0, :])
            cstb_sb = constp.tile([P, P], bf16)
            nc.sync.dma_start(out=cstb_sb[:], in_=cstb_d[:, :])

            b3r_sb = constp.tile([1, OUT], bf16)
            cstB_sb = constp.tile([P, 2 * MH], f32)
            dstr_sb = constp.tile([P, T_TILES, ch], bf16)
            w1_sb = constp.tile([P, MH * KD1 * P], bf16)
            w2_sb = constp.tile([P, MH * KH * P], bf16)
            w3_sb = constp.tile([P, KH * OUT], bf16)
            cstLN_sb = constp.tile([P, 3 * OUT + 1], f32)

            # w1 slices stream on the ScalarE DMA queue, concurrent with the
            # Sync queue's edge-chunk DMAs (each descriptor costs ~650ns of
            # issue time on its queue; two queues halve the serialization).
            for m in range(MH):
                nc.scalar.dma_start(
                    out=w1_sb[:, m * KD1 * P:(m + 1) * KD1 * P],
                    in_=w1_d[:, m * KD1 * P:(m + 1) * KD1 * P])

            nfT_ap = nfT_d[:, :].rearrange("(k p) n -> p k n", p=P)

            def emit_agg_subtile(t, aggT, s, split_dma=False):
                """Segment-sum of one 128-node tile into aggT[:, s*P:(s+1)*P].

                split_dma: stage the edge chunks as two independent tiles so the
                first matmuls start as soon as the first slice lands (tile-level
                dependency tracking would otherwise wait for the whole load)."""
                if split_dma and ch > 4:
                    h = 4
                    parts = [(0, h), (h, ch)]
                else:
                    parts = [(0, ch)]
                efs, ohs = [], []
                dsrc = dstr0_sb if t < groups[0][1] else dstr_sb
                for (c0, c1) in parts:
                    ef_t = efp.tile([P, c1 - c0, P], bf16, tag="ef", name="ef_t")
                    nc.sync.dma_start(out=ef_t[:], in_=efb_d[t, :, c0:c1, :])
                    oh_t = ohp.tile([P, c1 - c0, P], bf16, tag="oh", name="oh_t")
                    nc.vector.tensor_tensor(
                        out=oh_t[:],
                        in0=dsrc[:, t, c0:c1, None].to_broadcast([P, c1 - c0, P]),
                        in1=cstb_sb[:, None, 0:P].to_broadcast([P, c1 - c0, P]),
                        op=Alu.is_equal,
                    )
                    efs.append(ef_t)
                    ohs.append(oh_t)
                ps_a = psA.tile([P, P], f32, tag="psA")
                cc = 0
                for (c0, c1), ef_t, oh_t in zip(parts, efs, ohs):
                    for j in range(c1 - c0):
                        nc.tensor.matmul(
                            out=ps_a[:],
                            lhsT=ef_t[:, j, :],
                            rhs=oh_t[:, j, :],
                            start=(cc == 0),
                            stop=(cc == ch - 1),
                        )
                        cc += 1
                nc.scalar.copy(out=aggT[:, s * P:(s + 1) * P], in_=ps_a[0:EDGE_DIM, :])

            # group 0's aggregation up front (weight DMAs stream in behind it)
            agg_tiles = {}
            agg_tiles[0] = aggp.tile([EDGE_DIM, GMAX * P], bf16, tag="agg",
                                     name="aggT")
            for s in range(g0):
                emit_agg_subtile(groups[0][0] + s, agg_tiles[0], s,
                                 split_dma=(s == 0))

            # prefetch everything else on the Sync queue, in need-order:
            # nfx + b1/b2 (L1 g0) -> w2 (L2 g0) -> w3 / LN consts (L3 g0) ->
            # full dst table (g1 agg) -> b3 row (final group). All land well
            # before their first consumer.
            nfx0 = nfxp.tile([P, KD, GMAX * P], bf16, tag="nfx")
            nc.sync.dma_start(out=nfx0[:, :, 0:g0 * P], in_=nfT_ap[:, :, 0:g0 * P])
            nc.sync.dma_start(out=cstB_sb[:], in_=cstB_d[:, :])
            for m in range(MH):
                nc.sync.dma_start(
                    out=w2_sb[:, m * KH * P:(m + 1) * KH * P],
                    in_=w2_d[:, m * KH * P:(m + 1) * KH * P])
            nc.sync.dma_start(out=w3_sb[:], in_=w3_d[:, :])
            nc.sync.dma_start(out=cstLN_sb[:], in_=cstLN_d[:, :])
            nc.sync.dma_start(out=dstr_sb[:], in_=dstr_d[:, :, :])
            nc.sync.dma_start(out=b3r_sb[:], in_=cstb3_d[:, :])

            for gi, (tstart, g) in enumerate(groups):
                nt = g * P  # free-dim width of this super-tile
                n0 = tstart * P
                aggT = agg_tiles.pop(gi)
                last_group = gi == len(groups) - 1

                # ---- node features (pre-transposed on host) ----
                if gi == 0:
                    nfx = nfx0
                else:
                    nfx = nfxp.tile([P, KD, GMAX * P], bf16, tag="nfx")
                    nc.sync.dma_start(out=nfx[:, :, 0:nt],
                                      in_=nfT_ap[:, :, n0:n0 + nt])

                # ---- layer 1: h1T[m] = relu(W1.T @ xT + b1), x = [nf; agg] ----
                h1 = h1p.tile([P, KH, GMAX * P], bf16, tag="h1")
                for m in range(MH):
                    ps = psM.tile([P, GMAX * P], f32, tag="psM")
                    for k in range(KD):
                        nc.tensor.matmul(
                            out=ps[:, 0:nt],
                            lhsT=w1_sb[:, (m * KD1 + k) * P:(m * KD1 + k + 1) * P],
                            rhs=nfx[:, k, 0:nt],
                            start=(k == 0),
                            stop=False,
                        )
                    nc.tensor.matmul(
                        out=ps[:, 0:nt],
                        lhsT=w1_sb[0:EDGE_DIM, (m * KD1 + KD) * P:(m * KD1 + KD) * P + P],
                        rhs=aggT[:, 0:nt],
                        start=False,
                        stop=True,
                    )
                    nc.scalar.activation(
                        out=h1[:, m, 0:nt], in_=ps[:, 0:nt], func=Act.Relu,
                        bias=cstB_sb[:, m:m + 1],
                    )

                # ---- layer 2 ----
                h2 = h2p.tile([P, KH, GMAX * P], bf16, tag="h2")
                for m in range(MH):
                    ps = psM.tile([P, GMAX * P], f32, tag="psM")
                    for k in range(KH):
                        nc.tensor.matmul(
                            out=ps[:, 0:nt],
                            lhsT=w2_sb[:, (m * KH + k) * P:(m * KH + k + 1) * P],
                            rhs=h1[:, k, 0:nt],
                            start=(k == 0),
                            stop=(k == KH - 1),
                        )
                    nc.scalar.activation(
                        out=h2[:, m, 0:nt], in_=ps[:, 0:nt], func=Act.Relu,
                        bias=cstB_sb[:, MH + m:MH + m + 1],
                    )

                # ---- layer 3 (nodes on partitions) + LayerNorm ----
                # aggregation for group gi+1 interleaves here: its matmuls keep
                # TensorE fed while each subtile's LayerNorm chain drains.
                if gi + 1 < len(groups):
                    tstart_nx, g_nx = groups[gi + 1]
                    agg_tiles[gi + 1] = aggp.tile([EDGE_DIM, GMAX * P], bf16,
                                                  tag="agg", name="aggT")
                else:
                    tstart_nx, g_nx = 0, 0
                for s in range(max(g, g_nx)):
                    if s < g_nx:
                        emit_agg_subtile(tstart_nx + s, agg_tiles[gi + 1], s)
                    if s >= g:
                        continue
                    ps_y = psY.tile([P, OUT], f32, tag="psY")
                    if last_group:
                        # + b3 via a K=1 ones-matmul: keeps the bias add off
                        # the serial post-matmul tail of the final tile.
                        nc.tensor.matmul(
                            out=ps_y[:], lhsT=ones1[0:1, :], rhs=b3r_sb[0:1, :],
                            start=True, stop=False,
                        )
                    for k in range(KH):
                        nc.tensor.matmul(
                            out=ps_y[:],
                            lhsT=h2[:, k, s * P:(s + 1) * P],
                            rhs=w3_sb[:, k * OUT:(k + 1) * OUT],
                            start=(k == 0 and not last_group),
                            stop=(k == KH - 1),
                        )
                    if not last_group:
                        # + b3 (broadcast rows) on VectorE, off the TensorE
                        # critical path
                        nc.vector.tensor_tensor(
                            out=ps_y[:], in0=ps_y[:],
                            in1=cstLN_sb[:, 2 * OUT:3 * OUT],
                            op=Alu.add,
                        )
                    st6 = stp.tile([P, 6], f32, tag="st6")
                    nc.vector.bn_stats(st6[:], ps_y[:])
                    mv = stp.tile([P, 2], f32, tag="mv")
                    nc.vector.bn_aggr(mv[:], st6[:])
                    std = stp.tile([P, 1], f32, tag="std")
                    nc.scalar.activation(std[:], mv[:, 1:2], Act.Sqrt,
                                         bias=cstLN_sb[:, 3 * OUT:])
                    rstd = stp.tile([P, 1], f32, tag="rstd")
                    nc.vector.reciprocal(rstd[:], std[:])
                    nmr = stp.tile([P, 1], f32, tag="nmr")
                    nc.vector.tensor_scalar(
                        out=nmr[:], in0=mv[:, 0:1], scalar1=rstd[:], scalar2=-1.0,
                        op0=Alu.mult, op1=Alu.mult,
                    )
                    yn = yop.tile([P, OUT], f32, tag="yn")
                    nc.scalar.activation(
                        out=yn[:], in_=ps_y[:], func=Act.Identity,
                        bias=nmr[:], scale=rstd[:],
                    )
                    if apply_gamma_beta:
                        nc.vector.tensor_tensor(
                            out=yn[:], in0=yn[:],
                            in1=cstLN_sb[:, 0:OUT], op=Alu.mult,
                        )
                        nc.vector.tensor_tensor(
                            out=yn[:], in0=yn[:],
                            in1=cstLN_sb[:, OUT:2 * OUT], op=Alu.add,
                        )
                    r0 = (tstart + s) * P
                    nc.sync.dma_start(out=y_d[r0:r0 + P, :], in_=yn[:])

    nc.compile()
    return nc


# ----------------------------------------------------------------------------
# Host-side sharding / layout prep
# ----------------------------------------------------------------------------

def _prep_core(c, node_feat, edge_feat, dst, ch):
    lo = c * NPC
    sel = np.flatnonzero((dst >= lo) & (dst < lo + NPC))
    d = (dst[sel] - lo).astype(np.int64)
    order = np.argsort(d, kind="stable")
    sel = sel[order]
    d = d[order]
    tile_of = d >> 7
    counts = np.bincount(tile_of, minlength=T_TILES)
    offs = np.zeros(T_TILES, np.int64)
    np.cumsum(counts[:-1], out=offs[1:])
    rank = np.arange(d.size) - offs[tile_of]
    p_slot = rank % P
    c_slot = rank // P
    assert c_slot.max(initial=0) < ch

    efb = np.zeros((T_TILES, P, ch, P), BF16)
    efb[tile_of, p_slot, c_slot, :EDGE_DIM] = edge_feat[sel].astype(BF16)
    dstr = np.full((T_TILES, P, ch), -1.0, BF16)
    dstr[tile_of, p_slot, c_slot] = (d - (tile_of << 7)).astype(BF16)
    dstr = np.ascontiguousarray(dstr.transpose(1, 0, 2))

    nfT = np.zeros((NODE_DIM, NPAD), BF16)
    nfT[:, :NPC] = node_feat[lo:lo + NPC].T.astype(BF16)
    return {"efb": efb, "dstr": dstr, "nfT": nfT}


def _prep_shared(W1, b1, W2, b2, W3, b3, gamma, beta):
    KD1 = NODE_DIM // P + 1
    MH = HID // P
    KH = HID // P

    w1p = np.zeros((KD1 * P, HID), np.float32)
    w1p[:NODE_DIM + EDGE_DIM] = W1
    # m-major: col index (m*KD1 + k)*P + j
    w1 = np.ascontiguousarray(
        w1p.reshape(KD1, P, MH, P).transpose(1, 2, 0, 3)).reshape(P, -1).astype(BF16)
    w2 = np.ascontiguousarray(
        W2.reshape(KH, P, MH, P).transpose(1, 2, 0, 3)).reshape(P, -1).astype(BF16)
    w3 = np.ascontiguousarray(
        W3.reshape(KH, P, OUT).transpose(1, 0, 2)).reshape(P, -1).astype(BF16)

    cstB = np.ascontiguousarray(np.concatenate(
        [b1.reshape(MH, P).T, b2.reshape(MH, P).T], axis=1).astype(np.float32))
    cstLN = np.ascontiguousarray(np.concatenate([
        np.tile(gamma.reshape(1, OUT), (P, 1)),
        np.tile(beta.reshape(1, OUT), (P, 1)),
        np.tile(b3.reshape(1, OUT), (P, 1)),
        np.full((P, 1), LN_EPS, np.float32),
    ], axis=1).astype(np.float32))

    cstb = np.tile(np.arange(P, dtype=np.float32)[None, :], (P, 1)).astype(BF16)
    cstb3 = np.ascontiguousarray(b3.reshape(1, OUT)).astype(BF16)
    return {"w1": w1, "w2": w2, "w3": w3, "cstB": cstB, "cstLN": cstLN,
            "cstb": cstb, "cstb3": cstb3}


# ----------------------------------------------------------------------------
# Entry point
# ----------------------------------------------------------------------------

def _ensure_axon_hooks_importable():
    """bass_utils imports antenv.axon_hooks when tracing is requested (even via
    the BASS_TRACE env var); provide a no-op stub if the module is absent so
    that path degrades to trace-skipped instead of crashing."""
    try:
        import antenv.axon_hooks  # noqa: F401
    except Exception:
        import sys
        import types
        try:
            import antenv
        except Exception:
            return
        mod = types.ModuleType('antenv.axon_hooks')
        mod._hook = None
        mod.set_axon_ntff_profile_hook = lambda h: setattr(mod, '_hook', h)
        mod.get_axon_ntff_profile_hook = lambda: mod._hook
        sys.modules['antenv.axon_hooks'] = mod
        antenv.axon_hooks = mod


def kernel(node_feat, edge_feat, edge_index, n_nodes, W1, b1, W2, b2, W3, b3,
           gamma, beta, _want_trace=False):
    from concourse.bass_utils import run_bass_kernel_spmd
    _ensure_axon_hooks_importable()

    node_feat = np.asarray(node_feat, dtype=np.float32)
    edge_feat = np.asarray(edge_feat, dtype=np.float32)
    edge_index = np.asarray(edge_index)
    assert int(n_nodes) == N_NODES
    assert node_feat.shape == (N_NODES, NODE_DIM)
    assert edge_feat.shape == (N_EDGES, EDGE_DIM)

    dst = edge_index[1].astype(np.int64)

    # fixed per-tile edge capacity (multiple of 128), global across cores
    counts = np.bincount(dst, minlength=N_NODES)
    padded = np.zeros((NCORES, NPAD), np.int64)
    padded[:, :NPC] = counts.reshape(NCORES, NPC)
    max_tile = int(padded.reshape(NCORES, T_TILES, P).sum(axis=2).max())
    ch = max(1, -(-max_tile // P))

    gamma = np.asarray(gamma, dtype=np.float32)
    beta = np.asarray(beta, dtype=np.float32)
    apply_gb = not (np.all(gamma == 1.0) and np.all(beta == 0.0))

    key = (ch, apply_gb)
    if key not in _CACHE:
        _CACHE[key] = _build_program(ch, apply_gb)
    nc = _CACHE[key]

    shared = _prep_shared(
        np.asarray(W1, np.float32), np.asarray(b1, np.float32),
        np.asarray(W2, np.float32), np.asarray(b2, np.float32),
        np.asarray(W3, np.float32), np.asarray(b3, np.float32),
        gamma, beta)

    in_maps = []
    for c in range(NCORES):
        m = _prep_core(c, node_feat, edge_feat, dst, ch)
        m.update(shared)
        in_maps.append(m)

    res = run_bass_kernel_spmd(nc, in_maps, list(range(NCORES)), trace=_want_trace)

    y = np.concatenate([res.results[c]["y"][:NPC] for c in range(NCORES)], axis=0)
    out = np.ascontiguousarray(y, dtype=np.float32)
    if _want_trace:
        kernel.last_results = res
    return out


kernel.last_results = None
